# revision 1
# baseline (speedup 1.0000x reference)
"""Causal self-attention Trainium2 kernel (8 NeuronCores).

Problem: B=4, S=2048, D=1024, H=16, HD=64, fp32.
    q/k/v = x @ W{q,k,v}.T + b;  split heads;  causal softmax(q k^T/8) v;
    out = attn @ Wo.T + bo.

Sharding: DP=4 over batch x TP=2 over heads. Core c handles batch c//2 and
heads 8*(c%2)..8*(c%2)+7; it computes a partial output projection over its
8 heads' features. The host sums the two TP partials per batch (bo is fed
as zeros to tp=1 cores so it is added exactly once).

Per-core dataflow (all phases software-pipelined via interleaved emission):
  xT [D,S] (host-transposed, fp32r). q/k are produced feature-major
  (qT/kT [512,S] bf16) by matmul(lhsT=W_tile [d,e], rhs=xT [d,s]); v is
  produced token-major [S, 8, 65] bf16 with a ones column per head so the
  PV matmul accumulates attn^T [64,sq] AND the softmax denominator (row 64)
  in one PSUM tile.
  Attention per (head-pair, 512-query-block): scores are computed
  transposed, [sk=128, sq=512] per head, the two heads of a pair on
  disjoint PE row groups (rows 0-63 / 64-127) so their matmuls overlap in
  the array; one ScalarE exp covers both heads' scores [128,1024] (no
  max-subtraction: scores are O(1) here; fp32 exp never overflows).
  Causal masking zeroes invalid entries of diagonal tiles post-exp with a
  GpSimd affine_select (diagonal tiles are processed first so their longer
  chain hides under off-diagonal iterations).
  Normalization: denominator row -> partition-0 copy -> fast-reciprocal
  (custom DVE ops misread non-zero base partitions, hence the copy) ->
  DRAM-bounce DMA broadcast across 64 partitions -> DVE multiply.
  Out-projection: psO [sq=128, e=512] = sum_hp matmul(lhsT=attnT(fp32r),
  rhs=WoT(fp32r)) + bo via a DMA-broadcast tile; interleaved into the
  attention stream per query-block as its tiles finish.

Matmul dtypes: projections fp32r (2 cyc/row on HW, ~1.5e-4 err);
attention q/k/v/exp in bf16 (frees SBUF for deeper pipelining; final
rel err ~2e-3). PSUM accumulation is always fp32.
"""

import numpy as np

import concourse.bass as bass
import concourse.mybir as mybir
import concourse.tile as tile
from concourse import bacc
from concourse.bass_utils import run_bass_kernel_spmd

B, S, D, H, HD = 4, 2048, 1024, 16, 64
SCALE = HD ** -0.5
N_CORES = 8
HLOC = H // 2          # 8 heads per core
FEAT = HLOC * HD       # 512 features per core
NDT = D // 128         # 8 contraction tiles
NFT = FEAT // 128      # 4 feature tiles
NQB = S // 512         # 4 query blocks of 512
NST = S // 128         # 16 token tiles of 128

F32 = mybir.dt.float32
F32R = mybir.dt.float32r
BF16 = mybir.dt.bfloat16
EXP = mybir.ActivationFunctionType.Exp

_NC_CACHE = {}


def build_nc():
    if "nc" in _NC_CACHE:
        return _NC_CACHE["nc"]
    from contextlib import ExitStack
    from collections import deque
    nc = bacc.Bacc("TRN2", target_bir_lowering=False, debug=False)

    xT = nc.dram_tensor("xT", [D, S], F32R, kind="ExternalInput")
    wqT = nc.dram_tensor("wqT", [D, FEAT], F32R, kind="ExternalInput")
    wkT = nc.dram_tensor("wkT", [D, FEAT], F32R, kind="ExternalInput")
    wvT = nc.dram_tensor("wvT", [D, FEAT], F32R, kind="ExternalInput")
    bqT = nc.dram_tensor("bqT", [FEAT, 1], F32, kind="ExternalInput")
    bkT = nc.dram_tensor("bkT", [FEAT, 1], F32, kind="ExternalInput")
    bv = nc.dram_tensor("bv", [1, FEAT], F32, kind="ExternalInput")
    woT = nc.dram_tensor("woT", [FEAT, D], F32R, kind="ExternalInput")
    bo = nc.dram_tensor("bo", [1, D], F32, kind="ExternalInput")
    out_p = nc.dram_tensor("out_p", [S, D], F32, kind="ExternalOutput")

    with tile.TileContext(nc) as tc:
        with tc.tile_pool(name="ps", bufs=4, space="PSUM") as psp, \
             tc.tile_pool(name="ps2", bufs=2, space="PSUM") as psp2, \
             tc.tile_pool(name="consts", bufs=1) as cpool, \
             tc.tile_pool(name="qk", bufs=1) as qkp, \
             tc.tile_pool(name="vt", bufs=1) as vtp, \
             tc.tile_pool(name="atp", bufs=8) as atp, \
             tc.tile_pool(name="wop", bufs=1) as wop, \
             tc.tile_pool(name="osp", bufs=4) as osp, \
             tc.tile_pool(name="esp", bufs=6) as esp, \
             tc.tile_pool(name="recp", bufs=2) as recp, \
             tc.tile_pool(name="recd", bufs=4, space="DRAM") as recdp, \
             tc.tile_pool(name="bcp", bufs=2) as bcp:

            # ---- constants ----
            vone = cpool.tile([128, HLOC, 1], F32)
            nc.vector.memset(vone, 1.0)
            bvb = cpool.tile([128, FEAT], F32)
            nc.gpsimd.dma_start(out=bvb, in_=bv[:, :].to_broadcast([128, FEAT]))
            bob = cpool.tile([128, D], F32)
            nc.gpsimd.dma_start(out=bob, in_=bo[:, :].to_broadcast([128, D]))
            bq_sb = cpool.tile([128, NFT], F32)
            nc.sync.dma_start(
                out=bq_sb, in_=bqT[:, :].rearrange("(f p) o -> p (f o)", p=128))
            bk_sb = cpool.tile([128, NFT], F32)
            nc.sync.dma_start(
                out=bk_sb, in_=bkT[:, :].rearrange("(f p) o -> p (f o)", p=128))

            # ---- long-lived activation tiles (attention side in bf16) ----
            qt = [[qkp.tile([128, 512], BF16, name=f"qt{f}_{t}")
                   for t in range(NQB)] for f in range(NFT)]
            kt = [[qkp.tile([128, 512], BF16, name=f"kt{f}_{t}")
                   for t in range(NQB)] for f in range(NFT)]
            vt = [vtp.tile([128, HLOC, HD + 1], BF16, name=f"vt{st}")
                  for st in range(NST)]
            at = {}
            wo_sb = []

            def gen_load_wo():
                for hp in range(NFT):
                    woti = wop.tile([128, D], F32R, name=f"wo{hp}")
                    nc.scalar.dma_start(out=woti,
                                        in_=woT[128 * hp:128 * hp + 128, :])
                    wo_sb.append(woti)
                    yield

            # ================= projections (emitted interleaved) ==========
            proj_ctx = ExitStack()
            xtp = proj_ctx.enter_context(tc.tile_pool(name="xtp", bufs=16))
            wqkp = proj_ctx.enter_context(tc.tile_pool(name="wqk", bufs=1))
            wvp = proj_ctx.enter_context(tc.tile_pool(name="wvp", bufs=1))

            wtiles = {}

            def load_w(pname, wsrc, f):
                for d in range(NDT):
                    wti = wqkp.tile([128, 128], F32R, name=f"w{pname}{f}_{d}")
                    nc.scalar.dma_start(
                        out=wti,
                        in_=wsrc[128 * d:128 * d + 128, 128 * f:128 * f + 128])
                    wtiles[pname, f, d] = wti

            wv_sb = []

            def load_wv():
                for d in range(NDT):
                    wvt = wvp.tile([128, FEAT], F32R, name=f"wv{d}")
                    nc.scalar.dma_start(out=wvt,
                                        in_=wvT[128 * d:128 * d + 128, :])
                    wv_sb.append(wvt)

            def gen_proj_t4(t4):
                ts_ = slice(512 * t4, 512 * t4 + 512)
                xts = []
                for d in range(NDT):
                    xti = xtp.tile([128, 512], F32R, name=f"xt{t4}_{d}",
                                   tag="xt")
                    nc.sync.dma_start(out=xti,
                                      in_=xT[128 * d:128 * d + 128, ts_])
                    xts.append(xti)
                for pname, dst, bias, wsrc in (
                        ("q", qt, bq_sb, wqT), ("k", kt, bk_sb, wkT)):
                    for f in range(NFT):
                        if t4 == 0:
                            load_w(pname, wsrc, f)
                        ps = psp.tile([128, 512], F32, tag="ps",
                                      name=f"ps_{pname}{t4}_{f}")
                        for d in range(NDT):
                            nc.tensor.matmul(ps, wtiles[pname, f, d], xts[d],
                                             start=(d == 0),
                                             stop=(d == NDT - 1))
                        nc.vector.tensor_scalar_add(
                            dst[f][t4], ps, bias[:, f:f + 1])
                        yield
                if t4 == 0:
                    load_wv()
                for stl in range(4):
                    st = 4 * t4 + stl
                    ps = psp.tile([128, FEAT], F32, tag="ps", name=f"ps_v{st}")
                    for d in range(NDT):
                        nc.tensor.matmul(
                            ps, xts[d][:, 128 * stl:128 * stl + 128],
                            wv_sb[d], start=(d == 0), stop=(d == NDT - 1))
                    nc.vector.tensor_copy(vt[st][:, :, HD:HD + 1], vone)
                    nc.vector.tensor_add(
                        vt[st][:, :, 0:HD],
                        ps.rearrange("p (h c) -> p h c", c=HD),
                        bvb.rearrange("p (h c) -> p h c", c=HD))
                    yield

            # ================= attention + out-proj =======================
            def emit_group(hp, qb, psA):
                h0, h1 = 2 * hp, 2 * hp + 1
                nj = 4 * (qb + 1)
                # diagonal tiles first: their exp+mask chain latency hides
                # under the off-diagonal iterations that follow.
                js = list(range(4 * qb, nj)) + list(range(0, 4 * qb))
                for i, j in enumerate(js):
                    jt, jc = j // 4, 128 * (j % 4)
                    kslc = kt[hp][jt][:, jc:jc + 128]
                    ps2 = psp2.tile([128, 1024], F32, tag="ps2",
                                    name=f"s{hp}_{qb}_{j}")
                    nc.tensor.matmul(ps2[:, 0:512],
                                     kslc[0:64, :], qt[hp][qb][0:64, :],
                                     start=True, stop=True)
                    nc.tensor.matmul(ps2[:, 512:1024],
                                     kslc[64:128, :], qt[hp][qb][64:128, :],
                                     start=True, stop=True)
                    es2 = esp.tile([128, 1024], BF16, tag="es",
                                   name=f"e{hp}_{qb}_{j}")
                    nc.scalar.activation(es2, ps2, EXP, scale=SCALE)
                    jr = j - 4 * qb
                    if jr >= 0:
                        nc.gpsimd.affine_select(
                            out=es2, in_=es2,
                            compare_op=mybir.AluOpType.is_ge,
                            fill=0.0, base=-128 * jr,
                            pattern=[[0, 2], [1, 512]],
                            channel_multiplier=-1)
                    nc.tensor.matmul(psA[0], vt[j][:, h0, :], es2[:, 0:512],
                                     start=(i == 0), stop=(i == nj - 1))
                    nc.tensor.matmul(psA[1], vt[j][:, h1, :],
                                     es2[:, 512:1024],
                                     start=(i == 0), stop=(i == nj - 1))
                    yield
                at[hp, qb] = atp.tile([128, 512], F32R, tag="at",
                                      name=f"at{hp}_{qb}")
                for idx, h in enumerate((h0, h1)):
                    r0 = 64 * (h % 2)
                    den0 = recp.tile([1, 512], F32, tag="den0",
                                     name=f"dn{h}_{qb}")
                    nc.vector.tensor_copy(den0, psA[idx][HD:HD + 1, :])
                    rec = recp.tile([1, 512], F32, tag="rec",
                                    name=f"rec{h}_{qb}")
                    nc.vector.reciprocal_approx_fast(rec, den0)
                    rd = recdp.tile([1, 512], F32, tag="rd",
                                    name=f"rd{h}_{qb}")
                    nc.scalar.dma_start(out=rd, in_=rec)
                    bcast = bcp.tile([64, 512], F32, tag="bc",
                                     name=f"bc{h}_{qb}")
                    nc.scalar.dma_start(
                        out=bcast, in_=rd[:, :].to_broadcast([64, 512]))
                    nc.vector.tensor_mul(
                        at[hp, qb][r0:r0 + 64, :], psA[idx][0:HD, :], bcast)
                    yield

            def gen_outproj(qb4):
                for r4 in range(4):
                    st = 4 * qb4 + r4
                    for e in range(2):
                        es_ = slice(512 * e, 512 * e + 512)
                        psO = psp.tile([128, 512], F32, tag="ps",
                                       name=f"psO{st}_{e}")
                        for hp in range(NFT):
                            nc.tensor.matmul(
                                psO, at[hp, qb4][:, 128 * r4:128 * r4 + 128],
                                wo_sb[hp][:, es_],
                                start=(hp == 0), stop=(hp == NFT - 1))
                        osb = osp.tile([128, 512], F32, tag="osb",
                                       name=f"o{st}_{e}")
                        nc.vector.tensor_add(osb, psO, bob[:, es_])
                        nc.sync.dma_start(
                            out=out_p[128 * st:128 * st + 128, es_], in_=osb)
                        yield

            fillers = deque()

            def filler_step():
                while fillers:
                    if next(fillers[0], "done") == "done":
                        fillers.popleft()
                        continue
                    return True
                return False

            def drain_fillers():
                while filler_step():
                    pass

            def run_group(hp, qb):
                psA = [psp.tile([HD + 1, 512], F32, tag="ps",
                                name=f"pa{h}_{qb}")
                       for h in (2 * hp, 2 * hp + 1)]
                for _ in emit_group(hp, qb, psA):
                    filler_step()

            for _ in gen_proj_t4(0):
                pass
            for _ in gen_proj_t4(1):
                pass
            fillers.append(gen_load_wo())
            fillers.append(gen_proj_t4(2))
            for hp in range(NFT):
                run_group(hp, 0)
            fillers.append(gen_outproj(0))
            fillers.append(gen_proj_t4(3))
            for hp in range(NFT):
                run_group(hp, 1)
            fillers.append(gen_outproj(1))
            drain_fillers()
            proj_ctx.close()
            for hp in range(NFT):
                run_group(hp, 2)
            fillers.append(gen_outproj(2))
            for hp in range(NFT):
                run_group(hp, 3)
            fillers.append(gen_outproj(3))
            drain_fillers()
    nc.finalize()
    _NC_CACHE["nc"] = nc
    return nc


def make_in_maps(x, Wq, bq, Wk, bk, Wv, bv, Wo, bo):
    import ml_dtypes
    bf = ml_dtypes.bfloat16
    in_maps = []
    for c in range(N_CORES):
        b, tp = c // 2, c % 2
        sl = slice(FEAT * tp, FEAT * (tp + 1))
        in_maps.append({
            "xT": np.ascontiguousarray(x[b].T),
            "wqT": np.ascontiguousarray(Wq[sl].T),
            "wkT": np.ascontiguousarray(Wk[sl].T),
            "wvT": np.ascontiguousarray(Wv[sl].T),
            "bqT": np.ascontiguousarray(bq[sl][:, None]),
            "bkT": np.ascontiguousarray(bk[sl][:, None]),
            "bv": np.ascontiguousarray(bv[sl][None, :]),
            "woT": np.ascontiguousarray(Wo[:, sl].T),
            "bo": (bo[None, :] if tp == 0
                   else np.zeros((1, D), np.float32)),
        })
    return in_maps


def run(inputs, trace=False, trace_cores=None):
    nc = build_nc()
    in_maps = make_in_maps(
        inputs["x"], inputs["Wq"], inputs["bq"], inputs["Wk"], inputs["bk"],
        inputs["Wv"], inputs["bv"], inputs["Wo"], inputs["bo"])
    res = run_bass_kernel_spmd(nc, in_maps, list(range(N_CORES)),
                               trace=trace, trace_cores=trace_cores)
    out = np.empty((B, S, D), np.float32)
    for b in range(B):
        out[b] = res.results[2 * b]["out_p"] + res.results[2 * b + 1]["out_p"]
    return out, res


def kernel(**inputs) -> np.ndarray:
    out, _ = run(inputs, trace=False)
    return out



# revision 12
# speedup vs baseline: 1.0831x; 1.0831x over previous
"""Causal self-attention Trainium2 kernel (8 NeuronCores).

Problem: B=4, S=2048, D=1024, H=16, HD=64, fp32.
    q/k/v = x @ W{q,k,v}.T + b;  split heads;  causal softmax(q k^T/8) v;
    out = attn @ Wo.T + bo.

Sharding: DP=4 over batch x TP=2 over heads. Core c handles batch c//2 and
heads 8*(c%2)..8*(c%2)+7; it computes a partial output projection over its
8 heads' features. The host sums the two TP partials per batch (bo is fed
as zeros to tp=1 cores so it is added exactly once).

Per-core dataflow (all phases software-pipelined via interleaved emission):
  xT [D,S] (host-transposed, fp32r). q/k are produced feature-major
  (qT/kT [512,S] bf16) by matmul(lhsT=W_tile [d,e], rhs=xT [d,s]); v is
  produced token-major [S, 8, 65] bf16 with a ones column per head so the
  PV matmul accumulates attn^T [64,sq] AND the softmax denominator (row 64)
  in one PSUM tile.
  Attention per (head-pair, 512-query-block): scores are computed
  transposed, [sk=128, sq=512] per head, the two heads of a pair on
  disjoint PE row groups (rows 0-63 / 64-127) so their matmuls overlap in
  the array; one ScalarE exp covers both heads' scores [128,1024] (no
  max-subtraction: scores are O(1) here; fp32 exp never overflows).
  Causal masking zeroes invalid entries of diagonal tiles post-exp with a
  GpSimd affine_select (diagonal tiles are processed first so their longer
  chain hides under off-diagonal iterations).
  Normalization: denominator row -> partition-0 copy -> fast-reciprocal
  (custom DVE ops misread non-zero base partitions, hence the copy) ->
  DRAM-bounce DMA broadcast across 64 partitions -> DVE multiply.
  Out-projection: psO [sq=128, e=512] = sum_hp matmul(lhsT=attnT(fp32r),
  rhs=WoT(fp32r)) + bo via a DMA-broadcast tile; interleaved into the
  attention stream per query-block as its tiles finish.

Matmul dtypes: projections fp32r (2 cyc/row on HW, ~1.5e-4 err);
attention q/k/v/exp in bf16 (frees SBUF for deeper pipelining; final
rel err ~2e-3). PSUM accumulation is always fp32.
"""

import numpy as np

import concourse.bass as bass
import concourse.mybir as mybir
import concourse.tile as tile
from concourse import bacc
from concourse.bass_utils import run_bass_kernel_spmd

B, S, D, H, HD = 4, 2048, 1024, 16, 64
SCALE = HD ** -0.5
N_CORES = 8
HLOC = H // 2          # 8 heads per core
FEAT = HLOC * HD       # 512 features per core
NDT = D // 128         # 8 contraction tiles
NFT = FEAT // 128      # 4 feature tiles
NQB = S // 512         # 4 query blocks of 512
NST = S // 128         # 16 token tiles of 128

F32 = mybir.dt.float32
F32R = mybir.dt.float32r
BF16 = mybir.dt.bfloat16
EXP = mybir.ActivationFunctionType.Exp

_NC_CACHE = {}


def build_nc():
    if "nc" in _NC_CACHE:
        return _NC_CACHE["nc"]
    from contextlib import ExitStack
    from collections import deque
    nc = bacc.Bacc("TRN2", target_bir_lowering=False, debug=False)

    xT = nc.dram_tensor("xT", [D, S], BF16, kind="ExternalInput")
    wqT = nc.dram_tensor("wqT", [D, FEAT], BF16, kind="ExternalInput")
    wkT = nc.dram_tensor("wkT", [D, FEAT], BF16, kind="ExternalInput")
    wvT = nc.dram_tensor("wvT", [D, FEAT], BF16, kind="ExternalInput")
    bqT = nc.dram_tensor("bqT", [FEAT, 1], F32, kind="ExternalInput")
    bkT = nc.dram_tensor("bkT", [FEAT, 1], F32, kind="ExternalInput")
    bv = nc.dram_tensor("bv", [1, FEAT], F32, kind="ExternalInput")
    woT = nc.dram_tensor("woT", [FEAT, D], BF16, kind="ExternalInput")
    bo = nc.dram_tensor("bo", [1, D], F32, kind="ExternalInput")
    out_p = nc.dram_tensor("out_p", [S, D], F32, kind="ExternalOutput")

    with tile.TileContext(nc) as tc:
        with tc.tile_pool(name="ps", bufs=4, space="PSUM") as psp, \
             tc.tile_pool(name="ps2", bufs=2, space="PSUM") as psp2, \
             tc.tile_pool(name="consts", bufs=1) as cpool, \
             tc.tile_pool(name="qk", bufs=1) as qkp, \
             tc.tile_pool(name="vt", bufs=1) as vtp, \
             tc.tile_pool(name="atp", bufs=8) as atp, \
             tc.tile_pool(name="wop", bufs=1) as wop, \
             tc.tile_pool(name="osp", bufs=4) as osp, \
             tc.tile_pool(name="esp", bufs=6) as esp, \
             tc.tile_pool(name="recp", bufs=2) as recp, \
             tc.tile_pool(name="bcp", bufs=2) as bcp:

            # ---- constants ----
            vone = cpool.tile([128, HLOC, 1], F32)
            nc.vector.memset(vone, 1.0)
            bvb = cpool.tile([128, FEAT], F32)
            nc.gpsimd.dma_start(out=bvb, in_=bv[:, :].to_broadcast([128, FEAT]))
            bob = cpool.tile([128, D], F32)
            nc.gpsimd.dma_start(out=bob, in_=bo[:, :].to_broadcast([128, D]))
            bq_sb = cpool.tile([128, NFT], F32)
            nc.sync.dma_start(
                out=bq_sb, in_=bqT[:, :].rearrange("(f p) o -> p (f o)", p=128))
            bk_sb = cpool.tile([128, NFT], F32)
            nc.sync.dma_start(
                out=bk_sb, in_=bkT[:, :].rearrange("(f p) o -> p (f o)", p=128))

            # ---- long-lived activation tiles (attention side in bf16) ----
            qt = [[qkp.tile([128, 512], BF16, name=f"qt{f}_{t}")
                   for t in range(NQB)] for f in range(NFT)]
            kt = [[qkp.tile([128, 512], BF16, name=f"kt{f}_{t}")
                   for t in range(NQB)] for f in range(NFT)]
            vt = [vtp.tile([128, HLOC, HD + 1], BF16, name=f"vt{st}")
                  for st in range(NST)]
            at = {}
            wo_sb = []

            def gen_load_wo():
                for hp in range(NFT):
                    woti = wop.tile([128, D], BF16, name=f"wo{hp}")
                    nc.gpsimd.dma_start(out=woti,
                                        in_=woT[128 * hp:128 * hp + 128, :])
                    wo_sb.append(woti)
                    yield

            # ================= projections (emitted interleaved) ==========
            proj_ctx = ExitStack()
            xtp = proj_ctx.enter_context(tc.tile_pool(name="xtp", bufs=16))
            wqkp = proj_ctx.enter_context(tc.tile_pool(name="wqk", bufs=1))
            wvp = proj_ctx.enter_context(tc.tile_pool(name="wvp", bufs=1))

            wtiles = {}

            def load_w(pname, wsrc, d):
                wti = wqkp.tile([128, FEAT], BF16, name=f"w{pname}{d}")
                eng = nc.gpsimd if d % 2 == 0 else nc.sync
                eng.dma_start(out=wti, in_=wsrc[128 * d:128 * d + 128, :])
                for f in range(NFT):
                    wtiles[pname, f, d] = wti[:, 128 * f:128 * f + 128]

            wv_sb = []

            def load_wv():
                for d in range(NDT):
                    wvt = wvp.tile([128, FEAT], BF16, name=f"wv{d}")
                    eng = nc.gpsimd if d % 2 == 0 else nc.sync
                    eng.dma_start(out=wvt,
                                  in_=wvT[128 * d:128 * d + 128, :])
                    wv_sb.append(wvt)

            def gen_proj_t4(t4):
                ts_ = slice(512 * t4, 512 * t4 + 512)
                xts = []
                for d in range(NDT):
                    xti = xtp.tile([128, 512], BF16, name=f"xt{t4}_{d}",
                                   tag="xt")
                    nc.sync.dma_start(out=xti,
                                      in_=xT[128 * d:128 * d + 128, ts_])
                    xts.append(xti)
                for pname, dst, bias, wsrc in (
                        ("q", qt, bq_sb, wqT), ("k", kt, bk_sb, wkT)):
                    if t4 == 0:
                        for d in range(NDT):
                            load_w(pname, wsrc, d)
                    for f in range(NFT):
                        ps = psp.tile([128, 512], F32, tag="ps",
                                      name=f"ps_{pname}{t4}_{f}")
                        for d in range(NDT):
                            nc.tensor.matmul(ps, wtiles[pname, f, d], xts[d],
                                             start=(d == 0),
                                             stop=(d == NDT - 1))
                        nc.vector.tensor_scalar_add(
                            dst[f][t4], ps, bias[:, f:f + 1])
                        yield
                if t4 == 0:
                    load_wv()
                for stl in range(4):
                    st = 4 * t4 + stl
                    ps = psp.tile([128, FEAT], F32, tag="ps", name=f"ps_v{st}")
                    for d in range(NDT):
                        nc.tensor.matmul(
                            ps, xts[d][:, 128 * stl:128 * stl + 128],
                            wv_sb[d], start=(d == 0), stop=(d == NDT - 1))
                    nc.vector.tensor_copy(vt[st][:, :, HD:HD + 1], vone)
                    nc.vector.tensor_add(
                        vt[st][:, :, 0:HD],
                        ps.rearrange("p (h c) -> p h c", c=HD),
                        bvb.rearrange("p (h c) -> p h c", c=HD))
                    yield

            # ================= attention + out-proj =======================
            def emit_group(hp, qb, psA):
                h0, h1 = 2 * hp, 2 * hp + 1
                nj = 4 * (qb + 1)
                # diagonal tiles first: their exp+mask chain latency hides
                # under the off-diagonal iterations that follow.
                js = list(range(4 * qb, nj)) + list(range(0, 4 * qb))
                for i, j in enumerate(js):
                    jt, jc = j // 4, 128 * (j % 4)
                    kslc = kt[hp][jt][:, jc:jc + 128]
                    ps2 = psp2.tile([128, 1024], F32, tag="ps2",
                                    name=f"s{hp}_{qb}_{j}")
                    nc.tensor.matmul(ps2[:, 0:512],
                                     kslc[0:64, :], qt[hp][qb][0:64, :],
                                     start=True, stop=True)
                    nc.tensor.matmul(ps2[:, 512:1024],
                                     kslc[64:128, :], qt[hp][qb][64:128, :],
                                     start=True, stop=True)
                    es2 = esp.tile([128, 1024], BF16, tag="es",
                                   name=f"e{hp}_{qb}_{j}")
                    nc.scalar.activation(es2, ps2, EXP, scale=SCALE)
                    jr = j - 4 * qb
                    if jr >= 0:
                        nc.gpsimd.affine_select(
                            out=es2, in_=es2,
                            compare_op=mybir.AluOpType.is_ge,
                            fill=0.0, base=-128 * jr,
                            pattern=[[0, 2], [1, 512]],
                            channel_multiplier=-1)
                    nc.tensor.matmul(psA[0], vt[j][:, h0, :], es2[:, 0:512],
                                     start=(i == 0), stop=(i == nj - 1))
                    nc.tensor.matmul(psA[1], vt[j][:, h1, :],
                                     es2[:, 512:1024],
                                     start=(i == 0), stop=(i == nj - 1))
                    yield
                at[hp, qb] = atp.tile([128, 512], BF16, tag="at",
                                      name=f"at{hp}_{qb}")
                for idx, h in enumerate((h0, h1)):
                    r0 = 64 * (h % 2)
                    den0 = recp.tile([1, 512], F32, tag="den0",
                                     name=f"dn{h}_{qb}")
                    nc.vector.tensor_copy(den0, psA[idx][HD:HD + 1, :])
                    rec = recp.tile([1, 512], F32, tag="rec",
                                    name=f"rec{h}_{qb}")
                    nc.vector.reciprocal_approx_fast(rec, den0)
                    bcast = bcp.tile([64, 512], F32, tag="bc",
                                     name=f"bc{h}_{qb}")
                    nc.gpsimd.partition_broadcast(bcast, rec[:, :])
                    nc.vector.tensor_mul(
                        at[hp, qb][r0:r0 + 64, :], psA[idx][0:HD, :], bcast)
                    yield

            def gen_outproj(qb4):
                for r4 in range(4):
                    st = 4 * qb4 + r4
                    for e in range(2):
                        es_ = slice(512 * e, 512 * e + 512)
                        psO = psp.tile([128, 512], F32, tag="ps",
                                       name=f"psO{st}_{e}")
                        for hp in range(NFT):
                            nc.tensor.matmul(
                                psO, at[hp, qb4][:, 128 * r4:128 * r4 + 128],
                                wo_sb[hp][:, es_],
                                start=(hp == 0), stop=(hp == NFT - 1))
                        osb = osp.tile([128, 512], F32, tag="osb",
                                       name=f"o{st}_{e}")
                        nc.vector.tensor_add(osb, psO, bob[:, es_])
                        nc.sync.dma_start(
                            out=out_p[128 * st:128 * st + 128, es_], in_=osb)
                        yield

            fillers = deque()

            def filler_step():
                while fillers:
                    if next(fillers[0], "done") == "done":
                        fillers.popleft()
                        continue
                    return True
                return False

            def drain_fillers():
                while filler_step():
                    pass

            def run_group(hp, qb):
                psA = [psp.tile([HD + 1, 512], F32, tag="ps",
                                name=f"pa{h}_{qb}")
                       for h in (2 * hp, 2 * hp + 1)]
                for _ in emit_group(hp, qb, psA):
                    filler_step()

            for _ in gen_proj_t4(0):
                pass
            for _ in gen_proj_t4(1):
                pass
            fillers.append(gen_load_wo())
            fillers.append(gen_proj_t4(2))
            for hp in range(NFT):
                run_group(hp, 0)
            fillers.append(gen_outproj(0))
            fillers.append(gen_proj_t4(3))
            for hp in range(NFT):
                run_group(hp, 1)
            fillers.append(gen_outproj(1))
            drain_fillers()
            proj_ctx.close()
            for hp in range(NFT):
                run_group(hp, 2)
            fillers.append(gen_outproj(2))
            for hp in range(NFT):
                run_group(hp, 3)
            fillers.append(gen_outproj(3))
            drain_fillers()
    nc.finalize()
    _NC_CACHE["nc"] = nc
    return nc


def make_in_maps(x, Wq, bq, Wk, bk, Wv, bv, Wo, bo):
    import ml_dtypes
    bf = ml_dtypes.bfloat16
    in_maps = []
    for c in range(N_CORES):
        b, tp = c // 2, c % 2
        sl = slice(FEAT * tp, FEAT * (tp + 1))
        in_maps.append({
            "xT": np.ascontiguousarray(x[b].T.astype(bf)),
            "wqT": np.ascontiguousarray(Wq[sl].T.astype(bf)),
            "wkT": np.ascontiguousarray(Wk[sl].T.astype(bf)),
            "wvT": np.ascontiguousarray(Wv[sl].T.astype(bf)),
            "bqT": np.ascontiguousarray(bq[sl][:, None]),
            "bkT": np.ascontiguousarray(bk[sl][:, None]),
            "bv": np.ascontiguousarray(bv[sl][None, :]),
            "woT": np.ascontiguousarray(Wo[:, sl].T.astype(bf)),
            "bo": (bo[None, :] if tp == 0
                   else np.zeros((1, D), np.float32)),
        })
    return in_maps


def run(inputs, trace=False, trace_cores=None):
    nc = build_nc()
    in_maps = make_in_maps(
        inputs["x"], inputs["Wq"], inputs["bq"], inputs["Wk"], inputs["bk"],
        inputs["Wv"], inputs["bv"], inputs["Wo"], inputs["bo"])
    res = run_bass_kernel_spmd(nc, in_maps, list(range(N_CORES)),
                               trace=trace, trace_cores=trace_cores)
    out = np.empty((B, S, D), np.float32)
    for b in range(B):
        out[b] = res.results[2 * b]["out_p"] + res.results[2 * b + 1]["out_p"]
    return out, res


def kernel(**inputs) -> np.ndarray:
    out, _ = run(inputs, trace=False)
    return out



# revision 16
# speedup vs baseline: 1.2108x; 1.1179x over previous
"""Causal self-attention Trainium2 kernel (8 NeuronCores).

Problem: B=4, S=2048, D=1024, H=16, HD=64, fp32.
    q/k/v = x @ W{q,k,v}.T + b;  split heads;  causal softmax(q k^T/8) v;
    out = attn @ Wo.T + bo.

Sharding: DP=4 over batch x TP=2 over heads. Core c handles batch c//2 and
heads 8*(c%2)..8*(c%2)+7; it computes a partial output projection over its
8 heads' features. The host sums the two TP partials per batch (bo is fed
as zeros to tp=1 cores so it is added exactly once).

Everything runs in bf16 (inputs cast host-side; fp32 PSUM accumulation),
which keeps the PE at 1 col/cycle and halves DMA traffic; final rel err
~3.5e-3 vs the 2e-2 gate.

Per-core dataflow: xT [D,S] host-transposed. q/k are produced
feature-major (qT/kT [512,S]) by matmul(lhsT=W_tile, rhs=xT); v is
token-major [S, 8, 65] with a ones column per head so the PV matmul
accumulates attn^T [64,sq] AND the softmax denominator (row 64) in one
PSUM tile.
Attention per (head-pair, 512-query-block): scores computed transposed
[sk=128, sq<=512] per head, the two heads of a pair in disjoint PSUM
column ranges; one ScalarE exp covers both heads' scores (no
max-subtraction: scores are O(1); fp32 exp never overflows). Diagonal
tiles restrict the query range to the causally-needed suffix (scores,
exp, select, and for qb>0 the PV) — the upper-left masked triangle is
never computed. Causal masking zeroes remaining invalid entries of
diagonal tiles post-exp with GpSimd affine_selects.
Normalization: denominator row -> partition-0 copy -> fast-reciprocal
-> gpsimd partition_broadcast across 64 partitions -> DVE multiply.
Out-projection: psO [sq=128, e=512] = sum_hp matmul(lhsT=attnT,
rhs=WoT) + bo.

Scheduling: projections for t4>=2, the Wo load and the out-projections
are emitted as single-matmul-granularity "fillers" interleaved into the
attention stream, so the PE stays dense (avoiding HAM clock-down) while
ScalarE paces the exp chain.
"""

import numpy as np

import concourse.bass as bass
import concourse.mybir as mybir
import concourse.tile as tile
from concourse import bacc
from concourse.bass_utils import run_bass_kernel_spmd

B, S, D, H, HD = 4, 2048, 1024, 16, 64
SCALE = HD ** -0.5
N_CORES = 8
HLOC = H // 2          # 8 heads per core
FEAT = HLOC * HD       # 512 features per core
NDT = D // 128         # 8 contraction tiles
NFT = FEAT // 128      # 4 feature tiles
NQB = S // 512         # 4 query blocks of 512
NST = S // 128         # 16 token tiles of 128

F32 = mybir.dt.float32
BF16 = mybir.dt.bfloat16
EXP = mybir.ActivationFunctionType.Exp

_NC_CACHE = {}


def build_nc():
    if "nc" in _NC_CACHE:
        return _NC_CACHE["nc"]
    from contextlib import ExitStack
    from collections import deque
    nc = bacc.Bacc("TRN2", target_bir_lowering=False, debug=False)

    xT = nc.dram_tensor("xT", [D, S], BF16, kind="ExternalInput")
    wqT = nc.dram_tensor("wqT", [D, FEAT], BF16, kind="ExternalInput")
    wkT = nc.dram_tensor("wkT", [D, FEAT], BF16, kind="ExternalInput")
    wvT = nc.dram_tensor("wvT", [D, FEAT], BF16, kind="ExternalInput")
    bqT = nc.dram_tensor("bqT", [FEAT, 1], F32, kind="ExternalInput")
    bkT = nc.dram_tensor("bkT", [FEAT, 1], F32, kind="ExternalInput")
    bv = nc.dram_tensor("bv", [1, FEAT], F32, kind="ExternalInput")
    woT = nc.dram_tensor("woT", [FEAT, D], BF16, kind="ExternalInput")
    bo = nc.dram_tensor("bo", [1, D], F32, kind="ExternalInput")
    out_p = nc.dram_tensor("out_p", [S, D], F32, kind="ExternalOutput")

    with tile.TileContext(nc) as tc:
        with tc.tile_pool(name="ps", bufs=4, space="PSUM") as psp, \
             tc.tile_pool(name="ps2", bufs=2, space="PSUM") as psp2, \
             tc.tile_pool(name="consts", bufs=1) as cpool, \
             tc.tile_pool(name="qk", bufs=1) as qkp, \
             tc.tile_pool(name="vt", bufs=1) as vtp, \
             tc.tile_pool(name="atp", bufs=16) as atp, \
             tc.tile_pool(name="wop", bufs=1) as wop, \
             tc.tile_pool(name="osp", bufs=4) as osp, \
             tc.tile_pool(name="esp", bufs=6) as esp, \
             tc.tile_pool(name="recp", bufs=2) as recp, \
             tc.tile_pool(name="bcp", bufs=2) as bcp:

            # ---- constants (emitted lazily, near first use) ----
            consts = {}

            def load_bias_qk(pname, bsrc):
                t = cpool.tile([128, NFT], F32, name=f"b{pname}")
                nc.sync.dma_start(
                    out=t, in_=bsrc[:, :].rearrange("(f p) o -> p (f o)",
                                                    p=128))
                consts[pname] = t
                return t

            # ---- long-lived activation tiles ----
            qt = [[qkp.tile([128, 512], BF16, name=f"qt{f}_{t}")
                   for t in range(NQB)] for f in range(NFT)]
            kt = [[qkp.tile([128, 512], BF16, name=f"kt{f}_{t}")
                   for t in range(NQB)] for f in range(NFT)]
            vt = [vtp.tile([128, HLOC, HD + 1], BF16, name=f"vt{st}")
                  for st in range(NST)]
            at = {}
            wo_sb = []

            def gen_load_wo():
                bob = cpool.tile([128, D], F32, name="bob")
                nc.gpsimd.dma_start(out=bob,
                                    in_=bo[:, :].to_broadcast([128, D]))
                consts["bo"] = bob
                yield
                for hp in range(NFT):
                    woti = wop.tile([128, D], BF16, name=f"wo{hp}")
                    nc.gpsimd.dma_start(out=woti,
                                        in_=woT[128 * hp:128 * hp + 128, :])
                    wo_sb.append(woti)
                    yield

            # ================= projections (emitted interleaved) ==========
            proj_ctx = ExitStack()
            xtp = proj_ctx.enter_context(tc.tile_pool(name="xtp", bufs=16))
            wqkp = proj_ctx.enter_context(tc.tile_pool(name="wqk", bufs=1))
            wvp = proj_ctx.enter_context(tc.tile_pool(name="wvp", bufs=1))

            wtiles = {}

            def load_w(pname, wsrc, d):
                wti = wqkp.tile([128, FEAT], BF16, name=f"w{pname}{d}")
                eng = nc.gpsimd if d % 2 == 0 else nc.sync
                eng.dma_start(out=wti, in_=wsrc[128 * d:128 * d + 128, :])
                for f in range(NFT):
                    wtiles[pname, f, d] = wti[:, 128 * f:128 * f + 128]

            wv_sb = []

            def load_wv():
                vone = cpool.tile([128, HLOC, 1], F32, name="vone")
                nc.vector.memset(vone, 1.0)
                consts["vone"] = vone
                bvb = cpool.tile([128, FEAT], F32, name="bvb")
                nc.gpsimd.dma_start(out=bvb,
                                    in_=bv[:, :].to_broadcast([128, FEAT]))
                consts["bv"] = bvb
                for d in range(NDT):
                    wvt = wvp.tile([128, FEAT], BF16, name=f"wv{d}")
                    eng = nc.gpsimd if d % 2 == 0 else nc.sync
                    eng.dma_start(out=wvt,
                                  in_=wvT[128 * d:128 * d + 128, :])
                    wv_sb.append(wvt)

            def gen_proj_t4(t4):
                ts_ = slice(512 * t4, 512 * t4 + 512)
                xts = []
                for d in range(NDT):
                    xti = xtp.tile([128, 512], BF16, name=f"xt{t4}_{d}",
                                   tag="xt")
                    nc.sync.dma_start(out=xti,
                                      in_=xT[128 * d:128 * d + 128, ts_])
                    xts.append(xti)
                for pname, dst, wsrc, bsrc in (
                        ("q", qt, wqT, bqT), ("k", kt, wkT, bkT)):
                    if t4 == 0:
                        for d in range(NDT):
                            load_w(pname, wsrc, d)
                        load_bias_qk(pname, bsrc)
                        yield
                    for f in range(NFT):
                        ps = psp.tile([128, 512], F32, tag="ps",
                                      name=f"ps_{pname}{t4}_{f}")
                        for d in range(NDT):
                            nc.tensor.matmul(ps, wtiles[pname, f, d], xts[d],
                                             start=(d == 0),
                                             stop=(d == NDT - 1))
                            yield
                        nc.vector.tensor_scalar_add(
                            dst[f][t4], ps, consts[pname][:, f:f + 1])
                        yield
                if t4 == 0:
                    load_wv()
                    yield
                for stl in range(4):
                    st = 4 * t4 + stl
                    ps = psp.tile([128, FEAT], F32, tag="ps", name=f"ps_v{st}")
                    for d in range(NDT):
                        nc.tensor.matmul(
                            ps, xts[d][:, 128 * stl:128 * stl + 128],
                            wv_sb[d], start=(d == 0), stop=(d == NDT - 1))
                        yield
                    nc.vector.tensor_copy(vt[st][:, :, HD:HD + 1],
                                          consts["vone"])
                    nc.vector.tensor_add(
                        vt[st][:, :, 0:HD],
                        ps.rearrange("p (h c) -> p h c", c=HD),
                        consts["bv"].rearrange("p (h c) -> p h c", c=HD))
                    yield

            # ================= attention + out-proj =======================
            def emit_group(hp, qb, psA):
                h0, h1 = 2 * hp, 2 * hp + 1
                nj = 4 * (qb + 1)
                # diagonal tiles first: their exp+mask chain latency hides
                # under the off-diagonal iterations that follow.
                js = list(range(4 * qb, nj)) + list(range(0, 4 * qb))
                for i, j in enumerate(js):
                    jt, jc = j // 4, 128 * (j % 4)
                    kslc = kt[hp][jt][:, jc:jc + 128]
                    jr = j - 4 * qb
                    # c0: first causally-reachable query column for this
                    # key tile. Only restricted for qb>0 so the PSUM
                    # accumulation 'stop' lands on a full-width
                    # off-diagonal tile.
                    c0 = 128 * jr if (jr > 0 and qb > 0) else 0
                    ps2 = psp2.tile([128, 1024], F32, tag="ps2",
                                    name=f"s{hp}_{qb}_{j}")
                    nc.tensor.matmul(ps2[:, c0:512],
                                     kslc[0:64, :], qt[hp][qb][0:64, c0:512],
                                     start=True, stop=True)
                    nc.tensor.matmul(ps2[:, 512 + c0:1024],
                                     kslc[64:128, :],
                                     qt[hp][qb][64:128, c0:512],
                                     start=True, stop=True)
                    es2 = esp.tile([128, 1024], BF16, tag="es",
                                   name=f"e{hp}_{qb}_{j}")
                    nc.scalar.activation(es2[:, c0:1024], ps2[:, c0:1024],
                                         EXP, scale=SCALE)
                    if c0 > 0:
                        for cs in (slice(c0, 512), slice(512 + c0, 1024)):
                            nc.gpsimd.affine_select(
                                out=es2[:, cs], in_=es2[:, cs],
                                compare_op=mybir.AluOpType.is_ge,
                                fill=0.0, base=0,
                                pattern=[[1, 512 - c0]],
                                channel_multiplier=-1)
                    elif jr >= 0:
                        nc.gpsimd.affine_select(
                            out=es2, in_=es2,
                            compare_op=mybir.AluOpType.is_ge,
                            fill=0.0, base=-128 * jr,
                            pattern=[[0, 2], [1, 512]],
                            channel_multiplier=-1)
                    nc.tensor.matmul(psA[0][:, c0:512], vt[j][:, h0, :],
                                     es2[:, c0:512],
                                     start=(i == 0), stop=(i == nj - 1))
                    nc.tensor.matmul(psA[1][:, c0:512], vt[j][:, h1, :],
                                     es2[:, 512 + c0:1024],
                                     start=(i == 0), stop=(i == nj - 1))
                    yield
                at[hp, qb] = atp.tile([128, 512], BF16, tag="at",
                                      name=f"at{hp}_{qb}")
                for idx, h in enumerate((h0, h1)):
                    r0 = 64 * (h % 2)
                    den0 = recp.tile([1, 512], F32, tag="den0",
                                     name=f"dn{h}_{qb}")
                    nc.vector.tensor_copy(den0, psA[idx][HD:HD + 1, :])
                    rec = recp.tile([1, 512], F32, tag="rec",
                                    name=f"rec{h}_{qb}")
                    nc.vector.reciprocal_approx_fast(rec, den0)
                    bcast = bcp.tile([64, 512], F32, tag="bc",
                                     name=f"bc{h}_{qb}")
                    nc.gpsimd.partition_broadcast(bcast, rec[:, :])
                    nc.vector.tensor_mul(
                        at[hp, qb][r0:r0 + 64, :], psA[idx][0:HD, :], bcast)
                    yield

            def gen_outproj(qb4):
                for r4 in range(4):
                    st = 4 * qb4 + r4
                    for e in range(2):
                        es_ = slice(512 * e, 512 * e + 512)
                        psO = psp.tile([128, 512], F32, tag="ps",
                                       name=f"psO{st}_{e}")
                        for hp in range(NFT):
                            nc.tensor.matmul(
                                psO, at[hp, qb4][:, 128 * r4:128 * r4 + 128],
                                wo_sb[hp][:, es_],
                                start=(hp == 0), stop=(hp == NFT - 1))
                            yield
                        osb = osp.tile([128, 512], F32, tag="osb",
                                       name=f"o{st}_{e}")
                        nc.vector.tensor_add(osb, psO, consts["bo"][:, es_])
                        nc.sync.dma_start(
                            out=out_p[128 * st:128 * st + 128, es_], in_=osb)
                        yield

            fillers = deque()

            def filler_step():
                while fillers:
                    if next(fillers[0], "done") == "done":
                        fillers.popleft()
                        continue
                    return True
                return False

            def drain(gen):
                # finish a specific filler generator (and any queued
                # before it, to preserve deque order).
                while gen in fillers:
                    if not filler_step():
                        break

            def drain_fillers():
                while filler_step():
                    pass

            def run_group(hp, qb):
                psA = [psp.tile([HD + 1, 512], F32, tag="ps",
                                name=f"pa{h}_{qb}")
                       for h in (2 * hp, 2 * hp + 1)]
                for _ in emit_group(hp, qb, psA):
                    filler_step()

            for _ in gen_proj_t4(0):
                pass
            for _ in gen_proj_t4(1):
                pass
            g_wo = gen_load_wo()
            g_p2 = gen_proj_t4(2)
            g_p3 = gen_proj_t4(3)
            fillers.append(g_wo)
            fillers.append(g_p2)
            fillers.append(g_p3)
            for hp in range(NFT):
                run_group(hp, 0)
            for hp in range(NFT):
                run_group(hp, 1)
            drain(g_p2)
            fillers.append(gen_outproj(0))
            fillers.append(gen_outproj(1))
            for hp in range(NFT):
                run_group(hp, 2)
            drain(g_p3)
            fillers.append(gen_outproj(2))
            for hp in range(NFT):
                run_group(hp, 3)
            drain_fillers()
            proj_ctx.close()
            for _ in gen_outproj(3):
                pass
    nc.finalize()
    _NC_CACHE["nc"] = nc
    return nc


def make_in_maps(x, Wq, bq, Wk, bk, Wv, bv, Wo, bo):
    import ml_dtypes
    bf = ml_dtypes.bfloat16
    in_maps = []
    for c in range(N_CORES):
        b, tp = c // 2, c % 2
        sl = slice(FEAT * tp, FEAT * (tp + 1))
        in_maps.append({
            "xT": np.ascontiguousarray(x[b].T.astype(bf)),
            "wqT": np.ascontiguousarray(Wq[sl].T.astype(bf)),
            "wkT": np.ascontiguousarray(Wk[sl].T.astype(bf)),
            "wvT": np.ascontiguousarray(Wv[sl].T.astype(bf)),
            "bqT": np.ascontiguousarray(bq[sl][:, None]),
            "bkT": np.ascontiguousarray(bk[sl][:, None]),
            "bv": np.ascontiguousarray(bv[sl][None, :]),
            "woT": np.ascontiguousarray(Wo[:, sl].T.astype(bf)),
            "bo": (bo[None, :] if tp == 0
                   else np.zeros((1, D), np.float32)),
        })
    return in_maps


def run(inputs, trace=False, trace_cores=None):
    nc = build_nc()
    in_maps = make_in_maps(
        inputs["x"], inputs["Wq"], inputs["bq"], inputs["Wk"], inputs["bk"],
        inputs["Wv"], inputs["bv"], inputs["Wo"], inputs["bo"])
    res = run_bass_kernel_spmd(nc, in_maps, list(range(N_CORES)),
                               trace=trace, trace_cores=trace_cores)
    out = np.empty((B, S, D), np.float32)
    for b in range(B):
        out[b] = res.results[2 * b]["out_p"] + res.results[2 * b + 1]["out_p"]
    return out, res


def kernel(**inputs) -> np.ndarray:
    out, _ = run(inputs, trace=False)
    return out


# revision 24
# speedup vs baseline: 1.2657x; 1.0453x over previous
"""Causal self-attention Trainium2 kernel (8 NeuronCores).

Problem: B=4, S=2048, D=1024, H=16, HD=64, fp32.
    q/k/v = x @ W{q,k,v}.T + b;  split heads;  causal softmax(q k^T/8) v;
    out = attn @ Wo.T + bo.

Sharding: DP=4 over batch x TP=2 over heads. Core c handles batch c//2 and
heads 8*(c%2)..8*(c%2)+7; it computes a partial output projection over its
8 heads' features. The host sums the two TP partials per batch (bo is fed
as zeros to tp=1 cores so it is added exactly once).

Everything runs in bf16 (inputs cast host-side; fp32 PSUM accumulation),
which keeps the PE at 1 col/cycle and halves DMA traffic; final rel err
~3.5e-3 vs the 2e-2 gate.

Per-core dataflow: xT [D,S] host-transposed. q/k are produced
feature-major (qT/kT [512,S]) by matmul(lhsT=W_tile, rhs=xT); v is
token-major [S, 8, 65] with a ones column per head so the PV matmul
accumulates attn^T [64,sq] AND the softmax denominator (row 64) in one
PSUM tile.
Attention per (head-pair, 512-query-block): scores computed transposed
[sk=128, sq<=512] per head, the two heads of a pair in disjoint PSUM
column ranges; one ScalarE exp covers both heads' scores (no
max-subtraction: scores are O(1); fp32 exp never overflows). Diagonal
tiles restrict the query range to the causally-needed suffix (scores,
exp, select, and for qb>0 the PV) — the upper-left masked triangle is
never computed. Causal masking zeroes remaining invalid entries of
diagonal tiles post-exp with GpSimd affine_selects.
Normalization: denominator row -> partition-0 copy -> fast-reciprocal
-> gpsimd partition_broadcast across 64 partitions -> DVE multiply.
Out-projection: psO [sq=128, e=512] = sum_hp matmul(lhsT=attnT,
rhs=WoT) + bo.

Scheduling: projections for t4>=2, the Wo load and the out-projections
are emitted as single-matmul-granularity "fillers" interleaved into the
attention stream, so the PE stays dense (avoiding HAM clock-down) while
ScalarE paces the exp chain.
"""

import numpy as np

import concourse.bass as bass
import concourse.mybir as mybir
import concourse.tile as tile
from concourse import bacc
from concourse.bass_utils import run_bass_kernel_spmd

B, S, D, H, HD = 4, 2048, 1024, 16, 64
SCALE = HD ** -0.5
N_CORES = 8
HLOC = H // 2          # 8 heads per core
FEAT = HLOC * HD       # 512 features per core
NDT = D // 128         # 8 contraction tiles
NFT = FEAT // 128      # 4 feature tiles
NQB = S // 512         # 4 query blocks of 512
NST = S // 128         # 16 token tiles of 128

F32 = mybir.dt.float32
BF16 = mybir.dt.bfloat16
EXP = mybir.ActivationFunctionType.Exp

_NC_CACHE = {}


def build_nc():
    if "nc" in _NC_CACHE:
        return _NC_CACHE["nc"]
    from contextlib import ExitStack
    from collections import deque
    nc = bacc.Bacc("TRN2", target_bir_lowering=False, debug=False)

    xT = nc.dram_tensor("xT", [D, S], BF16, kind="ExternalInput")
    wqT = nc.dram_tensor("wqT", [D, FEAT], BF16, kind="ExternalInput")
    wkT = nc.dram_tensor("wkT", [D, FEAT], BF16, kind="ExternalInput")
    wvT = nc.dram_tensor("wvT", [D, FEAT], BF16, kind="ExternalInput")
    bqT = nc.dram_tensor("bqT", [FEAT, 1], F32, kind="ExternalInput")
    bkT = nc.dram_tensor("bkT", [FEAT, 1], F32, kind="ExternalInput")
    bv = nc.dram_tensor("bv", [1, FEAT], F32, kind="ExternalInput")
    woT = nc.dram_tensor("woT", [FEAT, D], BF16, kind="ExternalInput")
    bo = nc.dram_tensor("bo", [1, D], F32, kind="ExternalInput")
    out_p = nc.dram_tensor("out_p", [S, D], F32, kind="ExternalOutput")

    with tile.TileContext(nc) as tc:
        with tc.tile_pool(name="ps", bufs=2, space="PSUM") as psp, \
             tc.tile_pool(name="ps2", bufs=2, space="PSUM") as psp2, \
             tc.tile_pool(name="pa", bufs=2, space="PSUM") as pap, \
             tc.tile_pool(name="consts", bufs=1) as cpool, \
             tc.tile_pool(name="qk", bufs=1) as qkp, \
             tc.tile_pool(name="vt", bufs=1) as vtp, \
             tc.tile_pool(name="atp", bufs=16) as atp, \
             tc.tile_pool(name="wop", bufs=1) as wop, \
             tc.tile_pool(name="osp", bufs=4) as osp, \
             tc.tile_pool(name="esp", bufs=6) as esp, \
             tc.tile_pool(name="recp", bufs=2) as recp, \
             tc.tile_pool(name="bcp", bufs=2) as bcp:

            # ---- constants (emitted lazily, near first use) ----
            consts = {}

            def load_bias_qk(pname, bsrc):
                t = cpool.tile([128, NFT], F32, name=f"b{pname}")
                nc.sync.dma_start(
                    out=t, in_=bsrc[:, :].rearrange("(f p) o -> p (f o)",
                                                    p=128))
                consts[pname] = t
                return t

            # ---- long-lived activation tiles ----
            qt = [[qkp.tile([128, 512], BF16, name=f"qt{f}_{t}")
                   for t in range(NQB)] for f in range(NFT)]
            kt = [[qkp.tile([128, 512], BF16, name=f"kt{f}_{t}")
                   for t in range(NQB)] for f in range(NFT)]
            vt = [vtp.tile([128, HLOC, HD + 1], BF16, name=f"vt{st}")
                  for st in range(NST)]
            at = {}
            wo_sb = []

            def gen_load_wo():
                bob = cpool.tile([128, D], F32, name="bob")
                nc.gpsimd.dma_start(out=bob,
                                    in_=bo[:, :].to_broadcast([128, D]))
                consts["bo"] = bob
                yield
                for hp in range(NFT):
                    woti = wop.tile([128, D], BF16, name=f"wo{hp}")
                    nc.gpsimd.dma_start(out=woti,
                                        in_=woT[128 * hp:128 * hp + 128, :])
                    wo_sb.append(woti)
                    yield

            # ================= projections (emitted interleaved) ==========
            proj_ctx = ExitStack()
            xtp = proj_ctx.enter_context(tc.tile_pool(name="xtp", bufs=16))
            wqkp = proj_ctx.enter_context(tc.tile_pool(name="wqk", bufs=1))
            wvp = proj_ctx.enter_context(tc.tile_pool(name="wvp", bufs=1))

            wtiles = {}
            _rr = [0]
            _early_engs = (nc.scalar, nc.sync, nc.gpsimd)

            def early_eng():
                # round-robin DMA issues across the three DGE-capable
                # queues (scalar is free until the first exp).
                e = _early_engs[_rr[0] % 3]
                _rr[0] += 1
                return e

            def load_w(pname, wsrc, dp):
                # one DMA per pair of 128-row contraction tiles
                wti = wqkp.tile([128, 2, FEAT], BF16, name=f"w{pname}{dp}")
                early_eng().dma_start(
                    out=wti,
                    in_=wsrc[256 * dp:256 * dp + 256, :].rearrange(
                        "(a p) s -> p a s", a=2))
                for f in range(NFT):
                    wtiles[pname, f, 2 * dp] = wti[:, 0, 128 * f:128 * f + 128]
                    wtiles[pname, f, 2 * dp + 1] = \
                        wti[:, 1, 128 * f:128 * f + 128]

            wv_sb = []

            def load_wv():
                vone = cpool.tile([128, HLOC, 1], F32, name="vone")
                nc.vector.memset(vone, 1.0)
                consts["vone"] = vone
                bvb = cpool.tile([128, FEAT], F32, name="bvb")
                nc.gpsimd.dma_start(out=bvb,
                                    in_=bv[:, :].to_broadcast([128, FEAT]))
                consts["bv"] = bvb
                for dp in range(NDT // 2):
                    wvt = wvp.tile([128, 2, FEAT], BF16, name=f"wv{dp}")
                    early_eng().dma_start(
                        out=wvt,
                        in_=wvT[256 * dp:256 * dp + 256, :].rearrange(
                            "(a p) s -> p a s", a=2))
                    wv_sb.append(wvt[:, 0, :])
                    wv_sb.append(wvt[:, 1, :])

            def gen_proj_t4(t4):
                ts_ = slice(512 * t4, 512 * t4 + 512)
                xts = []
                for dp in range(NDT // 2):
                    xti = xtp.tile([128, 2, 512], BF16, name=f"xt{t4}_{dp}",
                                   tag="xt")
                    eng = (early_eng() if t4 < 2
                           else (nc.sync if dp % 2 == 0 else nc.gpsimd))
                    eng.dma_start(
                        out=xti,
                        in_=xT[256 * dp:256 * dp + 256, ts_].rearrange(
                            "(a p) s -> p a s", a=2))
                    xts.append(xti[:, 0, :])
                    xts.append(xti[:, 1, :])
                for pname, dst, wsrc, bsrc in (
                        ("q", qt, wqT, bqT), ("k", kt, wkT, bkT)):
                    if t4 == 0:
                        for dp in range(NDT // 2):
                            load_w(pname, wsrc, dp)
                        load_bias_qk(pname, bsrc)
                        yield
                    for f in range(NFT):
                        ps = psp.tile([128, 512], F32, tag="ps",
                                      name=f"ps_{pname}{t4}_{f}")
                        for d in range(NDT):
                            nc.tensor.matmul(ps, wtiles[pname, f, d], xts[d],
                                             start=(d == 0),
                                             stop=(d == NDT - 1))
                            yield
                        nc.vector.tensor_scalar_add(
                            dst[f][t4], ps, consts[pname][:, f:f + 1])
                        yield
                if t4 == 0:
                    load_wv()
                    yield
                for stl in range(4):
                    st = 4 * t4 + stl
                    ps = psp.tile([128, FEAT], F32, tag="ps", name=f"ps_v{st}")
                    for d in range(NDT):
                        nc.tensor.matmul(
                            ps, xts[d][:, 128 * stl:128 * stl + 128],
                            wv_sb[d], start=(d == 0), stop=(d == NDT - 1))
                        yield
                    nc.vector.tensor_copy(vt[st][:, :, HD:HD + 1],
                                          consts["vone"])
                    nc.vector.tensor_add(
                        vt[st][:, :, 0:HD],
                        ps.rearrange("p (h c) -> p h c", c=HD),
                        consts["bv"].rearrange("p (h c) -> p h c", c=HD))
                    yield

            # ================= attention + out-proj =======================
            # Staircase causal mask: MA[p, c] = (c >= p + 512), bf16.
            # MA[:, 512-128*jr : 1024-128*jr][p, q] = (q >= p + 128*jr),
            # the mask a diagonal key tile at offset jr needs.
            MA = cpool.tile([128, 1024], BF16, name="MA")

            def init_mask():
                nc.vector.memset(MA, 1.0)
                nc.gpsimd.affine_select(
                    out=MA, in_=MA, compare_op=mybir.AluOpType.is_ge,
                    fill=0.0, base=-512, pattern=[[1, 1024]],
                    channel_multiplier=-1)

            def emit_iters(hp, qb, psA):
                h0, h1 = 2 * hp, 2 * hp + 1
                nj = 4 * (qb + 1)
                # diagonal tiles first: their exp+mask chain latency hides
                # under the off-diagonal iterations that follow.
                js = list(range(4 * qb, nj)) + list(range(0, 4 * qb))
                for i, j in enumerate(js):
                    jt, jc = j // 4, 128 * (j % 4)
                    kslc = kt[hp][jt][:, jc:jc + 128]
                    jr = j - 4 * qb
                    # c0: first causally-reachable query column for this
                    # key tile. Only restricted for qb>0 so the PSUM
                    # accumulation 'stop' lands on a full-width
                    # off-diagonal tile.
                    c0 = 128 * jr if (jr > 0 and qb > 0) else 0
                    ps2 = psp2.tile([128, 1024], F32, tag="ps2",
                                    name=f"s{hp}_{qb}_{j}")
                    nc.tensor.matmul(ps2[:, c0:512],
                                     kslc[0:64, :], qt[hp][qb][0:64, c0:512],
                                     start=True, stop=True)
                    nc.tensor.matmul(ps2[:, 512 + c0:1024],
                                     kslc[64:128, :],
                                     qt[hp][qb][64:128, c0:512],
                                     start=True, stop=True)
                    es2 = esp.tile([128, 1024], BF16, tag="es",
                                   name=f"e{hp}_{qb}_{j}")
                    nc.scalar.activation(es2[:, c0:1024], ps2[:, c0:1024],
                                         EXP, scale=SCALE)
                    if jr >= 0:
                        mslc = MA[:, 512 - 128 * jr + c0:1024 - 128 * jr]
                        for cs in (slice(c0, 512), slice(512 + c0, 1024)):
                            nc.vector.tensor_mul(es2[:, cs], es2[:, cs],
                                                 mslc)
                    nc.tensor.matmul(psA[0][:, c0:512], vt[j][:, h0, :],
                                     es2[:, c0:512],
                                     start=(i == 0), stop=(i == nj - 1))
                    nc.tensor.matmul(psA[1][:, c0:512], vt[j][:, h1, :],
                                     es2[:, 512 + c0:1024],
                                     start=(i == 0), stop=(i == nj - 1))
                    yield

            def gen_norm(hp, qb, psA):
                h0, h1 = 2 * hp, 2 * hp + 1
                at[hp, qb] = atp.tile([128, 512], BF16, tag="at",
                                      name=f"at{hp}_{qb}")
                for idx, h in enumerate((h0, h1)):
                    r0 = 64 * (h % 2)
                    den0 = recp.tile([1, 512], F32, tag="den0",
                                     name=f"dn{h}_{qb}")
                    nc.vector.tensor_copy(den0, psA[idx][HD:HD + 1, :])
                    rec = recp.tile([1, 512], F32, tag="rec",
                                    name=f"rec{h}_{qb}")
                    nc.vector.reciprocal_approx_fast(rec, den0)
                    bcast = bcp.tile([64, 512], F32, tag="bc",
                                     name=f"bc{h}_{qb}")
                    nc.gpsimd.partition_broadcast(bcast, rec[:, :])
                    nc.vector.tensor_mul(
                        at[hp, qb][r0:r0 + 64, :], psA[idx][0:HD, :], bcast)
                    yield

            def gen_outproj(qb4):
                for r4 in range(4):
                    st = 4 * qb4 + r4
                    for e in range(2):
                        es_ = slice(512 * e, 512 * e + 512)
                        psO = psp.tile([128, 512], F32, tag="ps",
                                       name=f"psO{st}_{e}")
                        for hp in range(NFT):
                            nc.tensor.matmul(
                                psO, at[hp, qb4][:, 128 * r4:128 * r4 + 128],
                                wo_sb[hp][:, es_],
                                start=(hp == 0), stop=(hp == NFT - 1))
                            yield
                        osb = osp.tile([128, 512], F32, tag="osb",
                                       name=f"o{st}_{e}")
                        nc.vector.tensor_add(osb, psO, consts["bo"][:, es_])
                        nc.sync.dma_start(
                            out=out_p[128 * st:128 * st + 128, es_], in_=osb)
                        yield

            fillers = deque()

            def filler_step():
                while fillers:
                    if next(fillers[0], "done") == "done":
                        fillers.popleft()
                        continue
                    return True
                return False

            def drain(gen):
                # finish a specific filler generator (and any queued
                # before it, to preserve deque order).
                while gen in fillers:
                    if not filler_step():
                        break

            def drain_fillers():
                while filler_step():
                    pass

            def run_group(hp, qb):
                psA = [pap.tile([HD + 1, 512], F32, tag="pa",
                                name=f"pa{h}_{qb}")
                       for h in (2 * hp, 2 * hp + 1)]
                for _ in emit_iters(hp, qb, psA):
                    filler_step()
                for _ in gen_norm(hp, qb, psA):
                    filler_step()

            for _ in gen_proj_t4(0):
                pass
            init_mask()
            for _ in gen_proj_t4(1):
                pass
            g_wo = gen_load_wo()
            g_p2 = gen_proj_t4(2)
            g_p3 = gen_proj_t4(3)
            fillers.append(g_wo)
            fillers.append(g_p2)
            fillers.append(g_p3)
            for hp in range(NFT):
                run_group(hp, 0)
            for hp in range(NFT):
                run_group(hp, 1)
            drain(g_p2)
            fillers.append(gen_outproj(0))
            fillers.append(gen_outproj(1))
            for hp in range(NFT):
                run_group(hp, 2)
            drain(g_p3)
            fillers.append(gen_outproj(2))
            for hp in range(NFT):
                run_group(hp, 3)
            drain_fillers()
            proj_ctx.close()
            for _ in gen_outproj(3):
                pass
    nc.finalize()
    _NC_CACHE["nc"] = nc
    return nc


def make_in_maps(x, Wq, bq, Wk, bk, Wv, bv, Wo, bo):
    import ml_dtypes
    bf = ml_dtypes.bfloat16
    in_maps = []
    for c in range(N_CORES):
        b, tp = c // 2, c % 2
        sl = slice(FEAT * tp, FEAT * (tp + 1))
        in_maps.append({
            "xT": np.ascontiguousarray(x[b].T.astype(bf)),
            "wqT": np.ascontiguousarray(Wq[sl].T.astype(bf)),
            "wkT": np.ascontiguousarray(Wk[sl].T.astype(bf)),
            "wvT": np.ascontiguousarray(Wv[sl].T.astype(bf)),
            "bqT": np.ascontiguousarray(bq[sl][:, None]),
            "bkT": np.ascontiguousarray(bk[sl][:, None]),
            "bv": np.ascontiguousarray(bv[sl][None, :]),
            "woT": np.ascontiguousarray(Wo[:, sl].T.astype(bf)),
            "bo": (bo[None, :] if tp == 0
                   else np.zeros((1, D), np.float32)),
        })
    return in_maps


def run(inputs, trace=False, trace_cores=None):
    nc = build_nc()
    in_maps = make_in_maps(
        inputs["x"], inputs["Wq"], inputs["bq"], inputs["Wk"], inputs["bk"],
        inputs["Wv"], inputs["bv"], inputs["Wo"], inputs["bo"])
    res = run_bass_kernel_spmd(nc, in_maps, list(range(N_CORES)),
                               trace=trace, trace_cores=trace_cores)
    out = np.empty((B, S, D), np.float32)
    for b in range(B):
        out[b] = res.results[2 * b]["out_p"] + res.results[2 * b + 1]["out_p"]
    return out, res


def kernel(**inputs) -> np.ndarray:
    out, _ = run(inputs, trace=False)
    return out


# revision 25
# speedup vs baseline: 1.2805x; 1.0117x over previous
"""Causal self-attention Trainium2 kernel (8 NeuronCores).

Problem: B=4, S=2048, D=1024, H=16, HD=64, fp32.
    q/k/v = x @ W{q,k,v}.T + b;  split heads;  causal softmax(q k^T/8) v;
    out = attn @ Wo.T + bo.

Sharding: DP=4 over batch x TP=2 over heads. Core c handles batch c//2 and
heads 8*(c%2)..8*(c%2)+7; it computes a partial output projection over its
8 heads' features. The host sums the two TP partials per batch (bo is fed
as zeros to tp=1 cores so it is added exactly once).

Everything runs in bf16 (inputs cast host-side; fp32 PSUM accumulation),
which keeps the PE at 1 col/cycle and halves DMA traffic; final rel err
~3.5e-3 vs the 2e-2 gate.

Per-core dataflow: xT [D,S] host-transposed. q/k are produced
feature-major (qT/kT [512,S]) by matmul(lhsT=W_tile, rhs=xT); v is
token-major [S, 8, 65] with a ones column per head so the PV matmul
accumulates attn^T [64,sq] AND the softmax denominator (row 64) in one
PSUM tile.
Attention per (head-pair, 512-query-block): scores computed transposed
[sk=128, sq<=512] per head, the two heads of a pair in disjoint PSUM
column ranges; one ScalarE exp covers both heads' scores (no
max-subtraction: scores are O(1); fp32 exp never overflows). Diagonal
tiles restrict the query range to the causally-needed suffix (scores,
exp, select, and for qb>0 the PV) — the upper-left masked triangle is
never computed. Causal masking zeroes remaining invalid entries of
diagonal tiles post-exp with GpSimd affine_selects.
Normalization: denominator row -> partition-0 copy -> fast-reciprocal
-> gpsimd partition_broadcast across 64 partitions -> DVE multiply.
Out-projection: psO [sq=128, e=512] = sum_hp matmul(lhsT=attnT,
rhs=WoT) + bo.

Scheduling: projections for t4>=2, the Wo load and the out-projections
are emitted as single-matmul-granularity "fillers" interleaved into the
attention stream, so the PE stays dense (avoiding HAM clock-down) while
ScalarE paces the exp chain.
"""

import numpy as np

import concourse.bass as bass
import concourse.mybir as mybir
import concourse.tile as tile
from concourse import bacc
from concourse.bass_utils import run_bass_kernel_spmd

B, S, D, H, HD = 4, 2048, 1024, 16, 64
SCALE = HD ** -0.5
N_CORES = 8
HLOC = H // 2          # 8 heads per core
FEAT = HLOC * HD       # 512 features per core
NDT = D // 128         # 8 contraction tiles
NFT = FEAT // 128      # 4 feature tiles
NQB = S // 512         # 4 query blocks of 512
NST = S // 128         # 16 token tiles of 128

F32 = mybir.dt.float32
BF16 = mybir.dt.bfloat16
EXP = mybir.ActivationFunctionType.Exp

_NC_CACHE = {}


def build_nc():
    if "nc" in _NC_CACHE:
        return _NC_CACHE["nc"]
    from contextlib import ExitStack
    from collections import deque
    nc = bacc.Bacc("TRN2", target_bir_lowering=False, debug=False)

    xT = nc.dram_tensor("xT", [D, S], BF16, kind="ExternalInput")
    wqT = nc.dram_tensor("wqT", [D, FEAT], BF16, kind="ExternalInput")
    wkT = nc.dram_tensor("wkT", [D, FEAT], BF16, kind="ExternalInput")
    wvT = nc.dram_tensor("wvT", [D, FEAT], BF16, kind="ExternalInput")
    bqT = nc.dram_tensor("bqT", [FEAT, 1], F32, kind="ExternalInput")
    bkT = nc.dram_tensor("bkT", [FEAT, 1], F32, kind="ExternalInput")
    bv = nc.dram_tensor("bv", [1, FEAT], F32, kind="ExternalInput")
    woT = nc.dram_tensor("woT", [FEAT, D], BF16, kind="ExternalInput")
    bo = nc.dram_tensor("bo", [1, D], F32, kind="ExternalInput")
    out_p = nc.dram_tensor("out_p", [S, D], F32, kind="ExternalOutput")

    with tile.TileContext(nc) as tc:
        with tc.tile_pool(name="ps", bufs=2, space="PSUM") as psp, \
             tc.tile_pool(name="ps2", bufs=2, space="PSUM") as psp2, \
             tc.tile_pool(name="pa", bufs=2, space="PSUM") as pap, \
             tc.tile_pool(name="consts", bufs=1) as cpool, \
             tc.tile_pool(name="qk", bufs=1) as qkp, \
             tc.tile_pool(name="vt", bufs=1) as vtp, \
             tc.tile_pool(name="atp", bufs=16) as atp, \
             tc.tile_pool(name="wop", bufs=1) as wop, \
             tc.tile_pool(name="osp", bufs=4) as osp, \
             tc.tile_pool(name="esp", bufs=6) as esp, \
             tc.tile_pool(name="recp", bufs=2) as recp, \
             tc.tile_pool(name="bcp", bufs=2) as bcp:

            # ---- constants (emitted lazily, near first use) ----
            consts = {}

            def load_bias_qk(pname, bsrc):
                t = cpool.tile([128, NFT], F32, name=f"b{pname}")
                nc.sync.dma_start(
                    out=t, in_=bsrc[:, :].rearrange("(f p) o -> p (f o)",
                                                    p=128))
                consts[pname] = t
                return t

            # ---- long-lived activation tiles ----
            qt = [[qkp.tile([128, 512], BF16, name=f"qt{f}_{t}")
                   for t in range(NQB)] for f in range(NFT)]
            kt = [[qkp.tile([128, 512], BF16, name=f"kt{f}_{t}")
                   for t in range(NQB)] for f in range(NFT)]
            vt = [vtp.tile([128, HLOC, HD + 1], BF16, name=f"vt{st}")
                  for st in range(NST)]
            at = {}
            wo_sb = []

            def gen_load_wo():
                bob = cpool.tile([128, D], F32, name="bob")
                nc.gpsimd.dma_start(out=bob,
                                    in_=bo[:, :].to_broadcast([128, D]))
                consts["bo"] = bob
                yield
                for hp in range(NFT):
                    woti = wop.tile([128, D], BF16, name=f"wo{hp}")
                    nc.gpsimd.dma_start(out=woti,
                                        in_=woT[128 * hp:128 * hp + 128, :])
                    wo_sb.append(woti)
                    yield

            # ================= projections (emitted interleaved) ==========
            proj_ctx = ExitStack()
            xtp = proj_ctx.enter_context(tc.tile_pool(name="xtp", bufs=16))
            wqkp = proj_ctx.enter_context(tc.tile_pool(name="wqk", bufs=1))
            wvp = proj_ctx.enter_context(tc.tile_pool(name="wvp", bufs=1))

            wtiles = {}
            _rr = [0]
            _early_engs = (nc.scalar, nc.sync, nc.gpsimd)

            def early_eng():
                # round-robin DMA issues across the three DGE-capable
                # queues (scalar is free until the first exp).
                e = _early_engs[_rr[0] % 3]
                _rr[0] += 1
                return e

            def load_w(pname, wsrc, dp):
                # one DMA per pair of 128-row contraction tiles
                wti = wqkp.tile([128, 2, FEAT], BF16, name=f"w{pname}{dp}")
                early_eng().dma_start(
                    out=wti,
                    in_=wsrc[256 * dp:256 * dp + 256, :].rearrange(
                        "(a p) s -> p a s", a=2))
                for f in range(NFT):
                    wtiles[pname, f, 2 * dp] = wti[:, 0, 128 * f:128 * f + 128]
                    wtiles[pname, f, 2 * dp + 1] = \
                        wti[:, 1, 128 * f:128 * f + 128]

            wv_sb = []

            def load_wv():
                vone = cpool.tile([128, HLOC, 1], F32, name="vone")
                nc.vector.memset(vone, 1.0)
                consts["vone"] = vone
                bvb = cpool.tile([128, FEAT], F32, name="bvb")
                nc.gpsimd.dma_start(out=bvb,
                                    in_=bv[:, :].to_broadcast([128, FEAT]))
                consts["bv"] = bvb
                for dp in range(NDT // 2):
                    wvt = wvp.tile([128, 2, FEAT], BF16, name=f"wv{dp}")
                    early_eng().dma_start(
                        out=wvt,
                        in_=wvT[256 * dp:256 * dp + 256, :].rearrange(
                            "(a p) s -> p a s", a=2))
                    wv_sb.append(wvt[:, 0, :])
                    wv_sb.append(wvt[:, 1, :])

            def gen_proj_t4(t4):
                ts_ = slice(512 * t4, 512 * t4 + 512)
                xts = []
                for dp in range(NDT // 2):
                    xti = xtp.tile([128, 2, 512], BF16, name=f"xt{t4}_{dp}",
                                   tag="xt")
                    eng = (early_eng() if t4 < 2
                           else (nc.sync if dp % 2 == 0 else nc.gpsimd))
                    eng.dma_start(
                        out=xti,
                        in_=xT[256 * dp:256 * dp + 256, ts_].rearrange(
                            "(a p) s -> p a s", a=2))
                    xts.append(xti[:, 0, :])
                    xts.append(xti[:, 1, :])
                for pname, dst, wsrc, bsrc in (
                        ("q", qt, wqT, bqT), ("k", kt, wkT, bkT)):
                    if t4 == 0:
                        for dp in range(NDT // 2):
                            load_w(pname, wsrc, dp)
                        load_bias_qk(pname, bsrc)
                        yield
                    for f in range(NFT):
                        ps = psp.tile([128, 512], F32, tag="ps",
                                      name=f"ps_{pname}{t4}_{f}")
                        for d in range(NDT):
                            nc.tensor.matmul(ps, wtiles[pname, f, d], xts[d],
                                             start=(d == 0),
                                             stop=(d == NDT - 1))
                            yield
                        nc.vector.tensor_scalar_add(
                            dst[f][t4], ps, consts[pname][:, f:f + 1])
                        yield
                if t4 == 0:
                    load_wv()
                    yield
                for stl in range(4):
                    st = 4 * t4 + stl
                    ps = psp.tile([128, FEAT], F32, tag="ps", name=f"ps_v{st}")
                    for d in range(NDT):
                        nc.tensor.matmul(
                            ps, xts[d][:, 128 * stl:128 * stl + 128],
                            wv_sb[d], start=(d == 0), stop=(d == NDT - 1))
                        yield
                    nc.vector.tensor_copy(vt[st][:, :, HD:HD + 1],
                                          consts["vone"])
                    nc.vector.tensor_add(
                        vt[st][:, :, 0:HD],
                        ps.rearrange("p (h c) -> p h c", c=HD),
                        consts["bv"].rearrange("p (h c) -> p h c", c=HD))
                    yield

            # ================= attention + out-proj =======================
            # Staircase causal mask: MA[p, c] = (c >= p + 512), bf16.
            # MA[:, 512-128*jr : 1024-128*jr][p, q] = (q >= p + 128*jr),
            # the mask a diagonal key tile at offset jr needs.
            MA = cpool.tile([128, 1024], BF16, name="MA")

            def init_mask():
                nc.vector.memset(MA, 1.0)
                nc.gpsimd.affine_select(
                    out=MA, in_=MA, compare_op=mybir.AluOpType.is_ge,
                    fill=0.0, base=-512, pattern=[[1, 1024]],
                    channel_multiplier=-1)

            def emit_iters(hp, qb, psA):
                h0, h1 = 2 * hp, 2 * hp + 1
                nj = 4 * (qb + 1)
                # diagonal tiles first: their exp+mask chain latency hides
                # under the off-diagonal iterations that follow.
                js = list(range(4 * qb, nj)) + list(range(0, 4 * qb))
                for i, j in enumerate(js):
                    jt, jc = j // 4, 128 * (j % 4)
                    kslc = kt[hp][jt][:, jc:jc + 128]
                    jr = j - 4 * qb
                    # c0: first causally-reachable query column for this
                    # key tile. Only restricted for qb>0 so the PSUM
                    # accumulation 'stop' lands on a full-width
                    # off-diagonal tile.
                    c0 = 128 * jr if (jr > 0 and qb > 0) else 0
                    ps2 = psp2.tile([128, 1024], F32, tag="ps2",
                                    name=f"s{hp}_{qb}_{j}")
                    nc.tensor.matmul(ps2[:, c0:512],
                                     kslc[0:64, :], qt[hp][qb][0:64, c0:512],
                                     start=True, stop=True)
                    nc.tensor.matmul(ps2[:, 512 + c0:1024],
                                     kslc[64:128, :],
                                     qt[hp][qb][64:128, c0:512],
                                     start=True, stop=True)
                    es2 = esp.tile([128, 1024], BF16, tag="es",
                                   name=f"e{hp}_{qb}_{j}")
                    nc.scalar.activation(es2[:, c0:1024], ps2[:, c0:1024],
                                         EXP, scale=SCALE)
                    if jr >= 0:
                        mslc = MA[:, 512 - 128 * jr + c0:1024 - 128 * jr]
                        for cs in (slice(c0, 512), slice(512 + c0, 1024)):
                            nc.vector.tensor_mul(es2[:, cs], es2[:, cs],
                                                 mslc)
                    nc.tensor.matmul(psA[0][:, c0:512], vt[j][:, h0, :],
                                     es2[:, c0:512],
                                     start=(i == 0), stop=(i == nj - 1))
                    nc.tensor.matmul(psA[1][:, c0:512], vt[j][:, h1, :],
                                     es2[:, 512 + c0:1024],
                                     start=(i == 0), stop=(i == nj - 1))
                    yield

            def gen_norm(hp, qb, psA):
                h0, h1 = 2 * hp, 2 * hp + 1
                at[hp, qb] = atp.tile([128, 512], BF16, tag="at",
                                      name=f"at{hp}_{qb}")
                for idx, h in enumerate((h0, h1)):
                    r0 = 64 * (h % 2)
                    den0 = recp.tile([1, 512], F32, tag="den0",
                                     name=f"dn{h}_{qb}")
                    nc.vector.tensor_copy(den0, psA[idx][HD:HD + 1, :])
                    rec = recp.tile([1, 512], F32, tag="rec",
                                    name=f"rec{h}_{qb}")
                    nc.vector.reciprocal_approx_fast(rec, den0)
                    bcast = bcp.tile([64, 512], F32, tag="bc",
                                     name=f"bc{h}_{qb}")
                    nc.gpsimd.partition_broadcast(bcast, rec[:, :])
                    nc.vector.tensor_mul(
                        at[hp, qb][r0:r0 + 64, :], psA[idx][0:HD, :], bcast)
                    yield

            def gen_outproj(qb4):
                for r4 in range(4):
                    st = 4 * qb4 + r4
                    for e in range(2):
                        es_ = slice(512 * e, 512 * e + 512)
                        psO = psp.tile([128, 512], F32, tag="ps",
                                       name=f"psO{st}_{e}")
                        for hp in range(NFT):
                            nc.tensor.matmul(
                                psO, at[hp, qb4][:, 128 * r4:128 * r4 + 128],
                                wo_sb[hp][:, es_],
                                start=(hp == 0), stop=(hp == NFT - 1))
                            yield
                        osb = osp.tile([128, 512], F32, tag="osb",
                                       name=f"o{st}_{e}")
                        nc.vector.tensor_add(osb, psO, consts["bo"][:, es_])
                        nc.sync.dma_start(
                            out=out_p[128 * st:128 * st + 128, es_], in_=osb)
                        yield

            fillers = deque()

            def filler_step():
                while fillers:
                    if next(fillers[0], "done") == "done":
                        fillers.popleft()
                        continue
                    return True
                return False

            def drain(gen):
                # finish a specific filler generator (and any queued
                # before it, to preserve deque order).
                while gen in fillers:
                    if not filler_step():
                        break

            def drain_fillers():
                while filler_step():
                    pass

            def run_group(hp, qb):
                psA = [pap.tile([HD + 1, 512], F32, tag="pa",
                                name=f"pa{h}_{qb}")
                       for h in (2 * hp, 2 * hp + 1)]
                for i, _ in enumerate(emit_iters(hp, qb, psA)):
                    filler_step()
                    if i < 3:
                        # extra pulls early in the group: keep the PE fed
                        # while the first PV waits on the previous
                        # group's norm to release its psA slot.
                        filler_step()
                for _ in gen_norm(hp, qb, psA):
                    filler_step()

            for _ in gen_proj_t4(0):
                pass
            init_mask()
            for _ in gen_proj_t4(1):
                pass
            g_wo = gen_load_wo()
            g_p2 = gen_proj_t4(2)
            g_p3 = gen_proj_t4(3)
            fillers.append(g_wo)
            fillers.append(g_p2)
            fillers.append(g_p3)
            for hp in range(NFT):
                run_group(hp, 0)
            for hp in range(NFT):
                run_group(hp, 1)
            drain(g_p2)
            fillers.append(gen_outproj(0))
            fillers.append(gen_outproj(1))
            for hp in range(NFT):
                run_group(hp, 2)
            drain(g_p3)
            fillers.append(gen_outproj(2))
            for hp in range(NFT):
                run_group(hp, 3)
            drain_fillers()
            proj_ctx.close()
            for _ in gen_outproj(3):
                pass
    nc.finalize()
    _NC_CACHE["nc"] = nc
    return nc


def make_in_maps(x, Wq, bq, Wk, bk, Wv, bv, Wo, bo):
    import ml_dtypes
    bf = ml_dtypes.bfloat16
    in_maps = []
    for c in range(N_CORES):
        b, tp = c // 2, c % 2
        sl = slice(FEAT * tp, FEAT * (tp + 1))
        in_maps.append({
            "xT": np.ascontiguousarray(x[b].T.astype(bf)),
            "wqT": np.ascontiguousarray(Wq[sl].T.astype(bf)),
            "wkT": np.ascontiguousarray(Wk[sl].T.astype(bf)),
            "wvT": np.ascontiguousarray(Wv[sl].T.astype(bf)),
            "bqT": np.ascontiguousarray(bq[sl][:, None]),
            "bkT": np.ascontiguousarray(bk[sl][:, None]),
            "bv": np.ascontiguousarray(bv[sl][None, :]),
            "woT": np.ascontiguousarray(Wo[:, sl].T.astype(bf)),
            "bo": (bo[None, :] if tp == 0
                   else np.zeros((1, D), np.float32)),
        })
    return in_maps


def run(inputs, trace=False, trace_cores=None):
    nc = build_nc()
    in_maps = make_in_maps(
        inputs["x"], inputs["Wq"], inputs["bq"], inputs["Wk"], inputs["bk"],
        inputs["Wv"], inputs["bv"], inputs["Wo"], inputs["bo"])
    res = run_bass_kernel_spmd(nc, in_maps, list(range(N_CORES)),
                               trace=trace, trace_cores=trace_cores)
    out = np.empty((B, S, D), np.float32)
    for b in range(B):
        out[b] = res.results[2 * b]["out_p"] + res.results[2 * b + 1]["out_p"]
    return out, res


def kernel(**inputs) -> np.ndarray:
    out, _ = run(inputs, trace=False)
    return out
